# revision 18
# baseline (speedup 1.0000x reference)
"""CostVolumeLayer Trainium2 kernel.

Computes the local cost volume: for search_range R=4,
  out[b, di*9+dj, i, j] = sum_c src[b,c,i,j] * tgt_zp[b,c,i-2R+di, j-2R+dj]
(tgt zero-padded outside its bounds; the window is OFF-CENTER, covering
tgt rows i-8..i and cols j-8..j - faithful to the torch reference, whose
window indices index the zero-padded tensor directly and whose negative
indices wrap into the zero pad).

Strategy (8 NeuronCores, SPMD):
  - Shard: core c -> batch b = c//2, W-half wh = c%2 (cols 64*wh..64*wh+63).
    Each core gets src shard [C=128, 64, 64] (block-reordered) and a
    zero-padded tgt halo shard [C=128, 72, 72] in bf16 (host pre-pads the
    interior cols; the device memsets the 8 top halo rows).
  - Device: for each 8x16 pixel block, TWO M=64 bf16 matmuls - one per
    pixel-row half (mi in 0..3 / 4..7) - each streaming only its 12x24=288
    window band, both writing the same [128, 288] PSUM region at partition
    offsets 0/64 (bass auto-derives PE col-strip tile_position from the
    output base partition, so the two matmuls run concurrently).  Blocks
    are processed in pairs sharing a 2-bank PSUM tile; one fused
    full-128-partition fp32->fp16 copy per pair moves both bands to SBUF
    (alternating DVE/ACT), and each block group leaves as one plain DMA
    (groups alternate between the SP and ACT HWDGE rings so descriptor
    generation pipelines).
  - Host: zero-FLOP banded-diagonal gather from the band blocks into the
    [B, 81, H, W] output (the 81 needed entries per pixel live at
    n = ((mi%4)+di)*24 + (mj+dj), a per-partition-skewed pattern that
    engine access patterns cannot express on-chip).
"""

import numpy as np

R = 4
D = 2 * R + 1          # 9
B, C, H, W = 4, 128, 64, 128
NCORES = 8
WS = W // 2            # 64 cols per core shard
TH = H + 2 * R         # 72 padded tgt rows (R zero on top, R halo bottom)
TW = WS + 2 * R        # 72 padded tgt cols
BI, BJ = 8, 16         # pixel block: 8 rows x 16 cols = 128 = M
NBI, NBJ = H // BI, WS // BJ   # 8 x 4 = 32 blocks per core
HALF = 2               # pixel rows per matmul quarter (M = HALF*BJ = 32)
NQ = BI // HALF        # 4 matmuls per block (PE col-strip tiles)
WIN_I = HALF + 2 * R   # 10 window rows per quarter
WIN_J = BJ + 2 * R     # 24 window cols
BANDW = WIN_I * WIN_J  # 240 streamed columns per quarter-matmul
NBLK = NBI * NBJ       # 32
GRPS = [8, 8, 8, 4, 4]  # blocks per output DMA group (small tail groups
                        # overlap on the two HWDGE rings)
# input chunking by src block-row r: (row ranges) - fine-grained early,
# few chunks overall (each [128,*] DMA costs ~600ns of descriptor gen)
SCHUNKS = [(0, 1), (1, 2), (2, 4), (4, 6), (6, 8)]

_compiled = None


def _build_bass():
    import concourse.mybir as mybir
    from concourse import bacc
    from concourse.tile import TileContext

    f32 = mybir.dt.float32
    bf16 = mybir.dt.bfloat16
    fp16 = mybir.dt.float16
    nc = bacc.Bacc()
    # single combined input: [C, src block-reordered (64*64) ++ tgt payload
    # rows 8..72 of the padded [72, 72] shard (64*72)]
    SRCE = H * WS                  # 4096
    E = SRCE + (TH - 8) * TW
    inp = nc.dram_tensor("inp", [C, E], bf16, kind="ExternalInput")
    gouts = [nc.dram_tensor(f"gout{g}", [128, ng * BANDW], fp16,
                            kind="ExternalOutput") for g, ng in enumerate(GRPS)]

    with TileContext(nc) as tc:
        with (
            tc.tile_pool(name="inp", bufs=1) as inp_pool,
            tc.tile_pool(name="g", bufs=len(GRPS)) as gpool,
            tc.tile_pool(name="psum", bufs=3, space="PSUM") as psum_pool,
            tc.tile_pool(name="warmpsum", bufs=1, space="PSUM") as warm_pool,
        ):
            # src block-reordered [C, blk, 128 pixels]; tgt tile holds the
            # padded [72, 72] shard: rows 8..72 DMA-filled, rows 0..8 memset
            # (the top zero halo).
            a = inp_pool.tile([C, SRCE + TH * TW], bf16)
            s_v = a[:, :SRCE]
            t_v = a[:, SRCE:].rearrange("c (i j) -> c i j", j=TW)

            nc.vector.memset(t_v[:, 0:8, :], 0.0)

            # PE warm-up: a few dummy matmuls during the input-DMA wait push
            # the HAM clock gate toward 8/8; they finish right as the first
            # input chunk lands so they never delay real work.
            warm = inp_pool.tile([128, 512], bf16)
            nc.vector.memset(warm, 0.0)
            wps = warm_pool.tile([1, 512], f32)
            for _ in range(6):
                nc.tensor.matmul(wps, warm[:, :1], warm, start=True, stop=True)

            # Chunked input load in consumption order S0,T0,S1,T1,...
            # NO dependency chain: DMAs on one HWDGE ring drain FIFO in
            # issue order at full bandwidth and complete incrementally
            # (chaining inserts a ~2us sem-wait + descriptor-regen gap per
            # chunk, and chunks much under ~200KB make the ~600ns/DMA
            # descriptor generation the bandwidth limit).
            iv = inp.ap()
            for r0, r1 in SCHUNKS:
                so, so1 = r0 * NBJ * 128, r1 * NBJ * 128
                to, to1 = SRCE + r0 * 8 * TW, SRCE + r1 * 8 * TW
                nc.sync.dma_start(out=a[:, so:so1], in_=iv[:, so:so1])
                nc.sync.dma_start(out=a[:, 8 * TW + to:8 * TW + to1],
                                  in_=iv[:, to:to1])

            blk0 = 0
            for grp, ng in enumerate(GRPS):
                stage = gpool.tile([128, ng * BANDW], fp16)
                for pair in range(ng // 2):
                    # 2-bank PSUM tile: blocks at col 0 and 512 (bank 1)
                    psraw = psum_pool.tile([128, 1024], f32)
                    for k in range(2):
                        blk = blk0 + pair * 2 + k
                        bi, bj = divmod(blk, NBJ)
                        ps = psraw[:, 512 * k: 512 * k + BANDW]
                        for h in range(NQ):
                            lhsT = a[:, blk * 128 + 32 * h: blk * 128 + 32 * (h + 1)]
                            rhs = t_v[:, bi * BI + HALF * h: bi * BI + HALF * h + WIN_I,
                                      bj * BJ: bj * BJ + WIN_J]
                            # explicit tile_position: auto-derive rejects
                            # the 4th col strip (base partition 96)
                            nc.tensor.matmul(ps[32 * h:32 * (h + 1), :], lhsT, rhs,
                                             start=True, stop=True,
                                             tile_position=(0, 32 * h))
                    # fused copy of both blocks' bands, alternating engines
                    dst = stage[:, pair * 2 * BANDW: (pair * 2 + 2) * BANDW] \
                        .rearrange("p (b w) -> p b w", b=2)
                    src = psraw.rearrange("p (b w) -> p b w", b=2)[:, :, :BANDW]
                    if pair % 2 == 0:
                        nc.vector.tensor_copy(dst, src)
                    else:
                        nc.scalar.copy(dst, src)
                # outputs go via SWDGE (GpSimd): its descriptor generator
                # and queue rows are independent of the input HWDGE FIFO,
                # so output transfers start the moment their copies land
                # instead of queueing behind remaining input descriptors.
                nc.gpsimd.dma_start(out=gouts[grp].ap(), in_=stage)
                blk0 += ng
    nc.finalize()
    return nc


def _get_compiled():
    global _compiled
    if _compiled is None:
        _compiled = _build_bass()
    return _compiled


def _shard_inputs(src, tgt):
    """Build per-core input maps (host-side shard + zero-pad + bf16)."""
    import ml_dtypes

    bf16 = ml_dtypes.bfloat16
    in_maps = []
    for c in range(NCORES):
        b = c // 2
        w0 = WS * (c % 2)
        # block-reorder: [C, NBI, BI, NBJ, BJ] -> [C, (NBI NBJ), (BI BJ)]
        s = np.ascontiguousarray(
            src[b, :, :, w0:w0 + WS]
            .reshape(C, NBI, BI, NBJ, BJ)
            .transpose(0, 1, 3, 2, 4)
        ).reshape(C, H * WS)
        # tgt payload: padded rows 8..72 of the [72, 72] shard.  Padded
        # (q, x) holds tgt (q - 8, w0 + x - 8); the window for output pixel
        # (i, j_local) covers padded rows i..i+8, cols j_local..j_local+8
        # = tgt rows i-8..i, cols w0+j_local-8..w0+j_local (the off-center
        # reference window).
        tp = np.zeros((C, TH - 8, TW), dtype=np.float32)
        clo = max(w0 - 8, 0)
        chi = min(w0 + WS, W)
        tp[:, :, clo - (w0 - 8): clo - (w0 - 8) + (chi - clo)] = \
            tgt[b, :, :, clo:chi]
        inp = np.concatenate([s, tp.reshape(C, (TH - 8) * TW)], axis=1)
        in_maps.append({"inp": np.ascontiguousarray(inp.astype(bf16))})
    return in_maps


# host-side gather indices: out[k=(di,dj)] at pixel (mi,mj) of a block sits
# in quarter h = mi // 2 at band column n = ((mi%2)+di)*WIN_J + (mj+dj)
_mi = np.arange(BI)[:, None, None, None]
_mj = np.arange(BJ)[None, :, None, None]
_di = np.arange(D)[None, None, :, None]
_dj = np.arange(D)[None, None, None, :]
_NIDX = (((_mi % HALF) + _di) * WIN_J + (_mj + _dj)).reshape(BI, BJ, D * D)


def _unshard_output(results):
    out = np.empty((B, D * D, H, W), dtype=np.float32)
    for c in range(NCORES):
        b = c // 2
        w0 = WS * (c % 2)
        g = np.concatenate(
            [results[c][f"gout{i}"].astype(np.float32)
             .reshape(128, ng, BANDW).transpose(1, 0, 2)
             for i, ng in enumerate(GRPS)], axis=0
        ).reshape(NBI, NBJ, BI, BJ, BANDW)
        # gather: v[bi,bj,mi,mj,k] = g[bi,bj,mi,mj,_NIDX[mi,mj,k]]
        v = np.take_along_axis(g, _NIDX[None, None], axis=-1)
        # -> out[b, k, bi*8+mi, w0+bj*16+mj]
        v = v.transpose(4, 0, 2, 1, 3)  # [81, NBI, BI, NBJ, BJ]
        out[b, :, :, w0:w0 + WS] = v.reshape(D * D, H, WS)
    return out


def kernel(src, tgt):
    from concourse.bass_utils import run_bass_kernel_spmd

    src = np.asarray(src, dtype=np.float32)
    tgt = np.asarray(tgt, dtype=np.float32)
    nc = _get_compiled()
    in_maps = _shard_inputs(src, tgt)
    res = run_bass_kernel_spmd(nc, in_maps, core_ids=list(range(NCORES)))
    return _unshard_output(res.results)
